# revision 20
# baseline (speedup 1.0000x reference)
"""DeFUM dense-transformer kernel for 8x Trainium2 NeuronCores.

Sharding: data-parallel over batch B=64 -> 8 batch elements per core.
Each core runs the full network (semantic attention + 4 encoder layers)
on its shard; no collectives. Host does layout staging only (concat,
transpose, dtype cast); all FLOPs happen on device.

Math (per core, BL=8, N=256 tokens, D=768, H=12, DH=64, DFF=2048):
  x   = concat(ocr, obj)                      [2048, 768]  (token-major, f32)
  R   = |log dv_i - log dv_j| per batch       bias for semantic attention
  sem: q,k,v = x@Wq.T etc; att = softmax(qk/sqrt(D)+R); h = LN(x + att@v)
  4x post-norm encoder layers (MHA + exact-gelu FFN)
  out = h[:, :64, :]

Layouts: residual stream h kept f32, token-major [128, 16, 768] in SBUF,
updated in place. Matmul operands are cast to bf16 at PSUM eviction
(fp32 accumulation in PSUM). Weights are host-pre-transposed to [in, out]
so every matmul streams contiguous DMA; activation transposes (h -> h^T,
att -> att^T) run on the PE in bf16.
"""

import numpy as np
import ml_dtypes

import concourse.bass as bass
import concourse.mybir as mybir
import concourse.tile as tile
from concourse.bass_utils import run_bass_kernel_spmd
from concourse.masks import make_identity

B, N_OCR, N_OBJ, D, H, L, DFF = 64, 64, 192, 768, 12, 4, 2048
N = N_OCR + N_OBJ            # 256 tokens per batch element
DH = D // H                  # 64
NCORES = 8
BL = B // NCORES             # 8 batch elements per core
T = BL * N                   # 2048 tokens per core
TC = T // 128                # 16 token chunks
DC = D // 128                # 6 d chunks
FC = DFF // 128              # 16 dff chunks
EPS = 1e-5
AF = mybir.ActivationFunctionType
ALU = mybir.AluOpType
F32 = mybir.dt.float32
BF16 = mybir.dt.bfloat16


def _build(flags):
    """Build the SPMD Bass program for one core."""
    nc = bass.Bass()

    # ---- DRAM parameters (host-staged layouts) ----
    x_d = nc.declare_dram_parameter("x", [T, D], F32, isOutput=False)
    xT_d = nc.declare_dram_parameter("xT", [D, T], BF16, isOutput=False)
    dvr_d = nc.declare_dram_parameter("dv_rows", [128, BL, N], F32, isOutput=False)
    dvc_d = nc.declare_dram_parameter("dv_cols", [128, 2 * BL], F32, isOutput=False)
    wqkvT_d = nc.declare_dram_parameter("wqkvT", [L + 1, D, 3 * D], BF16, False)
    woT_d = nc.declare_dram_parameter("woT", [L, D, D], BF16, False)
    w1T_d = nc.declare_dram_parameter("w1T", [L, D, DFF], BF16, False)
    w2T_d = nc.declare_dram_parameter("w2T", [L, DFF, D], BF16, False)
    qkb_d = nc.declare_dram_parameter("qkb_cols", [L + 1, 128, 12], F32, False)
    vb_d = nc.declare_dram_parameter("vb_cols", [L + 1, 128, 6], F32, False)
    w1b_d = nc.declare_dram_parameter("w1b_cols", [L, 128, FC], F32, False)
    # free-dim bias/ln rows (used only when the matching flag is set)
    brow_d = nc.declare_dram_parameter("bias_rows", [1 + 2 * L, D], F32, False)
    lng_d = nc.declare_dram_parameter("ln_g", [1 + 2 * L, D], F32, False)
    lnb_d = nc.declare_dram_parameter("ln_b", [1 + 2 * L, D], F32, False)
    out_d = nc.declare_dram_parameter("out", [BL, N_OCR, D], F32, isOutput=True)

    def bcast_ap(src_ap, parts=128):
        """Broadcast a 1-partition AP across `parts` partitions (stride 0)."""
        return bass.AP(
            tensor=src_ap.tensor,
            offset=src_ap.offset,
            ap=[[0, parts]] + list(src_ap.ap),
        )

    with tile.TileContext(nc) as tc_:
        _emit(nc, tc_, flags, locals())
    _split_multi_waits(nc)
    return nc


def _split_multi_waits(nc):
    """This toolchain's walrus accepts at most ONE semaphore wait per
    instruction (64B ISA has a single wait-event slot and this build refuses
    to legalize). Hoist extra waits emitted by Tile onto injected NOPs on the
    same engine, placed immediately before the instruction — the engine
    stalls on the NOPs first, so semantics are identical."""
    import bass_rust

    n_new = 0
    for bb in nc.main_func.blocks:
        out = []
        changed = False
        for inst in bb.instructions:
            si = inst.sync_info
            waits = list(si.on_wait) if si is not None and si.on_wait else []
            if len(waits) > 1:
                changed = True
                for w in waits[:-1]:
                    nop = bass_rust.InstNoOp(name=f"I-wsplit-{n_new}")
                    n_new += 1
                    nop.engine = inst.engine
                    nop.sync_info = bass_rust.SyncInfo(on_wait=[w], on_update=[])
                    nc.register_instruction(nop)
                    out.append(nop)
                inst.sync_info = bass_rust.SyncInfo(
                    on_wait=[waits[-1]],
                    on_update=list(si.on_update) if si.on_update else [],
                )
            out.append(inst)
        if changed:
            bb.instructions = out
    return n_new


def _emit(nc, tc, flags, d):
    x_d, xT_d, dvr_d, dvc_d = d["x_d"], d["xT_d"], d["dvr_d"], d["dvc_d"]
    wqkvT_d, woT_d, w1T_d, w2T_d = d["wqkvT_d"], d["woT_d"], d["w1T_d"], d["w2T_d"]
    qkb_d, vb_d, w1b_d = d["qkb_d"], d["vb_d"], d["w1b_d"]
    brow_d, lng_d, lnb_d, out_d = d["brow_d"], d["lng_d"], d["lnb_d"], d["out_d"]
    bcast_ap = d["bcast_ap"]

    from contextlib import ExitStack
    ctx = ExitStack()
    const = ctx.enter_context(tc.tile_pool(name="const", bufs=1))
    wpool = ctx.enter_context(tc.tile_pool(name="w", bufs=1))
    act = ctx.enter_context(tc.tile_pool(name="act", bufs=1))
    wk1 = ctx.enter_context(tc.tile_pool(name="wk1", bufs=1))
    wk2 = ctx.enter_context(tc.tile_pool(name="wk2", bufs=2))
    ps = ctx.enter_context(tc.tile_pool(name="ps", bufs=3, space="PSUM"))
    psav = ctx.enter_context(tc.tile_pool(name="psav", bufs=2, space="PSUM"))
    pst = ctx.enter_context(tc.tile_pool(name="pst", bufs=1, space="PSUM"))
    psum_s = ctx.enter_context(tc.tile_pool(name="psum_s", bufs=2, space="PSUM"))

    # ---- constants ----
    ident = const.tile([128, 128], BF16)
    make_identity(nc, ident)
    ones_bf = const.tile([128, 128], BF16)
    nc.vector.memset(ones_bf, 1.0)
    eps_t = const.tile([128, 1], F32)
    nc.vector.memset(eps_t, EPS)
    qkb_sb = const.tile([128, L + 1, 12], F32)
    nc.sync.dma_start(qkb_sb, qkb_d[:].rearrange("l p c -> p l c"))
    vb_sb = const.tile([128, L + 1, 6], F32)
    nc.sync.dma_start(vb_sb, vb_d[:].rearrange("l p c -> p l c"))
    w1b_sb = const.tile([128, L, FC], F32)
    nc.sync.dma_start(w1b_sb, w1b_d[:].rearrange("l p c -> p l c"))
    ldvc = const.tile([128, 2 * BL], F32)
    nc.sync.dma_start(ldvc, dvc_d[:])
    nc.scalar.activation(ldvc, ldvc, AF.Ln)
    # log(dv) rows, pre-broadcast across partitions on host
    ldvr_all = const.tile([128, BL, N], F32)
    nc.sync.dma_start(ldvr_all, dvr_d[:])
    nc.scalar.activation(ldvr_all, ldvr_all, AF.Ln)

    # residual stream, in-place across all phases
    h = act.tile([128, TC, D], F32)
    nc.sync.dma_start(h, x_d[:].rearrange("(c p) d -> p c d", p=128))

    # ---- helpers ----
    def ln_chunk(c, g_bc=None, b_bc=None):
        """In-place LayerNorm of h[:, c, :] along the free (D) axis."""
        stats = wk2.tile([128, 3, 6], F32, tag="bnst")
        for i in range(3):
            nc.vector.bn_stats(stats[:, i, :], h[:, c, 256 * i:256 * (i + 1)])
        mv = wk2.tile([128, 2], F32, tag="bnmv")
        nc.vector.bn_aggr(mv, stats)
        # rstd = exp(-0.5*ln(var+eps)): Ln and Exp share one ACT table,
        # avoiding Sqrt-table reloads between softmax and LN
        rs = wk2.tile([128, 1], F32, tag="bnrs")
        nc.scalar.activation(rs, mv[:, 1:2], AF.Ln, bias=eps_t)
        nc.scalar.activation(rs, rs, AF.Exp, scale=-0.5)
        nc.vector.tensor_scalar(
            out=h[:, c, :], in0=h[:, c, :],
            scalar1=mv[:, 0:1], scalar2=rs,
            op0=ALU.subtract, op1=ALU.mult,
        )
        if g_bc is not None:
            nc.vector.tensor_mul(h[:, c, :], h[:, c, :], g_bc)
        if b_bc is not None:
            nc.vector.tensor_add(h[:, c, :], h[:, c, :], b_bc)

    def load_brow(tag, src_ap):
        t = wk2.tile([128, D], F32, tag=tag)
        nc.gpsimd.dma_start(t, bcast_ap(src_ap))
        return t

    def softmax_attT(ps_scT, b, li, R=None):
        """Transposed-softmax: scores^T psum [128(j), 2(jc), 256(i)] ->
        normalized att^T bf16 [128, 2, 256]. Row sums over j (the partition
        dim) via a ones-matrix matmul, which also broadcasts the sums across
        all 128 partitions; one DVE divide normalizes."""
        expT = wk2.tile([128, 2, N], BF16, tag="expT")
        scale = 1.0 / np.sqrt(np.float32(D if li == 0 else DH))
        if R is not None:
            lg = wk1.tile([128, 2, N], F32, tag="logit")
            nc.vector.scalar_tensor_tensor(
                out=lg, in0=ps_scT[:, :, :], scalar=float(scale),
                in1=R[:, :, :], op0=ALU.mult, op1=ALU.add,
            )
            nc.scalar.activation(expT[:, :, :], lg, AF.Exp)
        else:
            nc.scalar.activation(expT[:, :, :], ps_scT[:, :, :], AF.Exp,
                                 scale=float(scale))
        ps_sums = psum_s.tile([128, N], F32, tag="sums")
        for jc in range(2):
            nc.tensor.matmul(ps_sums, ones_bf, expT[:, jc, :],
                             start=(jc == 0), stop=(jc == 1))
        rec = wk2.tile([128, N], F32, tag="rsum")
        nc.vector.reciprocal(rec, ps_sums)
        rec_bc = bass.AP(
            tensor=rec.tensor, offset=rec.offset,
            ap=[list(rec.ap)[0], [0, 2], list(rec.ap)[1]],
        )
        attT = wk2.tile([128, 2, N], BF16, tag="attT")
        nc.vector.tensor_tensor(out=attT[:, :, :], in0=expT[:, :, :],
                                in1=rec_bc, op=ALU.mult)
        return attT

    # ================= semantic attention (layer index 0 in wqkvT) ========
    wqkv = wpool.tile([128, DC, 3 * D], BF16, tag="wqkv")
    nc.sync.dma_start(wqkv, wqkvT_d[0].rearrange("(c p) o -> p c o", p=128))

    sem_vb_bc = load_brow("sem_vb", brow_d[0]) if flags["sem_vb"] else None
    ln0_g_bc = load_brow("lng", lng_d[0]) if flags["lngb"] else None
    ln0_b_bc = load_brow("lnb", lnb_d[0]) if flags["lngb"] else None

    for b in range(BL):
        xT_b = wk1.tile([128, DC, N], BF16, tag="hT")
        nc.sync.dma_start(
            xT_b, xT_d[:].rearrange("(c p) t -> p c t", p=128)[:, :, N * b:N * (b + 1)])
        # q^T, k^T  [e-chunk, token] ; v [token, e]
        qkT = wk2.tile([128, 12, N], BF16, tag="qkT")
        for op2 in range(6):
            pq = ps.tile([128, 512], F32, tag="mm")
            for half in range(2):
                oc = 2 * op2 + half
                for dc in range(DC):
                    nc.tensor.matmul(pq[:, N * half:N * (half + 1)],
                                     wqkv[:, dc, 128 * oc:128 * (oc + 1)],
                                     xT_b[:, dc, :], start=(dc == 0),
                                     stop=(dc == DC - 1))
            if flags["qkb"]:
                for half in range(2):
                    oc = 2 * op2 + half
                    nc.vector.tensor_scalar_add(qkT[:, oc, :],
                                                pq[:, N * half:N * (half + 1)],
                                                qkb_sb[:, 0, oc:oc + 1])
            else:
                nc.vector.tensor_copy(qkT[:, 2 * op2:2 * op2 + 2, :], pq)
        v_b = wk2.tile([128, 2, D], BF16, tag="v")
        for tc2 in range(2):
            for ev in range(2):
                pv = ps.tile([128, 512], F32, tag="mm")
                for dc in range(DC):
                    nc.tensor.matmul(
                        pv[:, :384],
                        xT_b[:, dc, 128 * tc2:128 * (tc2 + 1)],
                        wqkv[:, dc, 2 * D + 384 * ev:2 * D + 384 * (ev + 1)],
                        start=(dc == 0), stop=(dc == DC - 1))
                nc.vector.tensor_copy(v_b[:, tc2, 384 * ev:384 * (ev + 1)],
                                      pv[:, :384])
        # R bias
        R = wk1.tile([128, 2, N], F32, tag="R")
        for ic in range(2):
            nc.vector.tensor_scalar(
                out=R[:, ic, :], in0=ldvr_all[:, b, :],
                scalar1=ldvc[:, 2 * b + ic:2 * b + ic + 1],
                scalar2=None, op0=ALU.subtract)
            nc.scalar.activation(R[:, ic, :], R[:, ic, :], AF.Abs)
        # transposed scores: scT[j, i] = sum_e kT[e,j] qT[e,i]
        ps_scT = ps.tile([128, 2, N], F32, tag="mm")
        for jc in range(2):
            for dc in range(DC):
                nc.tensor.matmul(ps_scT[:, jc, :],
                                 qkT[:, 6 + dc, 128 * jc:128 * (jc + 1)],
                                 qkT[:, dc, :],
                                 start=(dc == 0), stop=(dc == DC - 1))
        attT = softmax_attT(ps_scT, b, 0, R=R)
        # av (canonical [token, e]) + residual into h
        for ic in range(2):
            for ev in range(2):
                pa = ps.tile([128, 512], F32, tag="mm")
                for jc in range(2):
                    nc.tensor.matmul(pa[:, :384],
                                     attT[:, jc, 128 * ic:128 * (ic + 1)],
                                     v_b[:, jc, 384 * ev:384 * (ev + 1)],
                                     start=(jc == 0), stop=(jc == 1))
                c = 2 * b + ic
                sl = slice(384 * ev, 384 * (ev + 1))
                nc.vector.tensor_add(h[:, c, sl], pa[:, :384], h[:, c, sl])
                if sem_vb_bc is not None:
                    nc.vector.tensor_add(h[:, c, sl], h[:, c, sl], sem_vb_bc[:, sl])
        for ic in range(2):
            ln_chunk(2 * b + ic, ln0_g_bc, ln0_b_bc)

    # ================= encoder layers =================
    for li in range(L):
        wqkv = wpool.tile([128, DC, 3 * D], BF16, tag="wqkv")
        nc.sync.dma_start(wqkv, wqkvT_d[li + 1].rearrange("(c p) o -> p c o", p=128))
        wo = wpool.tile([128, DC, D], BF16, tag="wo")
        nc.sync.dma_start(wo, woT_d[li].rearrange("(c p) o -> p c o", p=128))
        w1 = wpool.tile([128, DC, DFF], BF16, tag="w1")
        nc.sync.dma_start(w1, w1T_d[li].rearrange("(c p) o -> p c o", p=128))
        w2 = wpool.tile([128, FC, D], BF16, tag="w2")
        nc.sync.dma_start(w2, w2T_d[li].rearrange("(c p) o -> p c o", p=128))

        wob_bc = load_brow("wob", brow_d[1 + 2 * li]) if flags["wob"] else None
        w2b_bc = load_brow("w2b", brow_d[2 + 2 * li]) if flags["w2b"] else None
        g1_bc = load_brow("lng", lng_d[1 + 2 * li]) if flags["lngb"] else None
        b1_bc = load_brow("lnb", lnb_d[1 + 2 * li]) if flags["lngb"] else None
        g2_bc = load_brow("lng2", lng_d[2 + 2 * li]) if flags["lngb"] else None
        b2_bc = load_brow("lnb2", lnb_d[2 + 2 * li]) if flags["lngb"] else None

        # ---- attention, one batch element at a time ----
        for b in range(BL):
            hcast = wk1.tile([128, 4, D], BF16, tag="hcast")
            for ic in range(2):
                nc.vector.tensor_copy(hcast[:, ic, :], h[:, 2 * b + ic, :])
            hT_b = wk1.tile([128, DC, N], BF16, tag="hT")
            for dc in range(DC):
                tp = pst.tile([128, 512], BF16, tag="tr")
                for ic in range(2):
                    nc.tensor.transpose(tp[:, 128 * ic:128 * (ic + 1)],
                                        hcast[:, ic, 128 * dc:128 * (dc + 1)], ident)
                nc.vector.tensor_copy(hT_b[:, dc, :], tp[:, :N])
            qkT = wk2.tile([128, 12, N], BF16, tag="qkT")
            for op2 in range(6):
                pq = ps.tile([128, 512], F32, tag="mm")
                for half in range(2):
                    oc = 2 * op2 + half
                    for dc in range(DC):
                        nc.tensor.matmul(pq[:, N * half:N * (half + 1)],
                                         wqkv[:, dc, 128 * oc:128 * (oc + 1)],
                                         hT_b[:, dc, :], start=(dc == 0),
                                         stop=(dc == DC - 1))
                if flags["qkb"]:
                    for half in range(2):
                        oc = 2 * op2 + half
                        nc.vector.tensor_scalar_add(
                            qkT[:, oc, :], pq[:, N * half:N * (half + 1)],
                            qkb_sb[:, li + 1, oc:oc + 1])
                else:
                    nc.vector.tensor_copy(qkT[:, 2 * op2:2 * op2 + 2, :], pq)
            v_b = wk2.tile([128, 2, D], BF16, tag="v")
            for tc2 in range(2):
                for ev in range(2):
                    pv = ps.tile([128, 512], F32, tag="mm")
                    for dc in range(DC):
                        nc.tensor.matmul(
                            pv[:, :384],
                            hT_b[:, dc, 128 * tc2:128 * (tc2 + 1)],
                            wqkv[:, dc, 2 * D + 384 * ev:2 * D + 384 * (ev + 1)],
                            start=(dc == 0), stop=(dc == DC - 1))
                    nc.vector.tensor_copy(v_b[:, tc2, 384 * ev:384 * (ev + 1)],
                                          pv[:, :384])
            # per-head attention; accumulate attn_out^T [e-chunk, token]
            aoT = wk2.tile([128, DC, N], BF16, tag="aoT")
            for ec in range(DC):
                pav = psav.tile([128, N], F32, tag="av")
                for sub in range(2):
                    hd, off = 2 * ec + sub, sub * 64
                    ps_scT = ps.tile([128, 2, N], F32, tag="mm")
                    for jc in range(2):
                        nc.tensor.matmul(
                            ps_scT[:, jc, :],
                            qkT[off:off + 64, 6 + ec, 128 * jc:128 * (jc + 1)],
                            qkT[off:off + 64, ec, :],
                            start=True, stop=True)
                    attT = softmax_attT(ps_scT, b, li + 1)
                    for jc in range(2):
                        nc.tensor.matmul(
                            pav[off:off + 64, :],
                            v_b[:, jc, DH * hd:DH * (hd + 1)],
                            attT[:, jc, :],
                            start=(jc == 0), stop=(jc == 1))
                nc.vector.tensor_scalar_add(aoT[:, ec, :], pav,
                                            vb_sb[:, li + 1, ec:ec + 1])
            # out-projection + residual into h, then LN1
            for ic in range(2):
                c = 2 * b + ic
                for fh in range(2):
                    po = ps.tile([128, 512], F32, tag="mm")
                    for ec in range(DC):
                        nc.tensor.matmul(po[:, :384],
                                         aoT[:, ec, 128 * ic:128 * (ic + 1)],
                                         wo[:, ec, 384 * fh:384 * (fh + 1)],
                                         start=(ec == 0), stop=(ec == DC - 1))
                    sl = slice(384 * fh, 384 * (fh + 1))
                    nc.vector.tensor_add(h[:, c, sl], po[:, :384], h[:, c, sl])
                    if wob_bc is not None:
                        nc.vector.tensor_add(h[:, c, sl], h[:, c, sl], wob_bc[:, sl])
                ln_chunk(c, g1_bc, b1_bc)

        # ---- FFN over 512-token chunks ----
        for nt in range(4):
            h1cast = wk1.tile([128, 4, D], BF16, tag="hcast")
            for t4 in range(4):
                nc.vector.tensor_copy(h1cast[:, t4, :], h[:, 4 * nt + t4, :])
            h1T = wk1.tile([128, DC, 512], BF16, tag="h1T")
            for dc in range(DC):
                tp = pst.tile([128, 512], BF16, tag="tr")
                for t4 in range(4):
                    nc.tensor.transpose(tp[:, 128 * t4:128 * (t4 + 1)],
                                        h1cast[:, t4, 128 * dc:128 * (dc + 1)], ident)
                nc.vector.tensor_copy(h1T[:, dc, :], tp)
            gT = wk1.tile([128, FC, 512], BF16, tag="gT")
            for fc in range(FC):
                pf = ps.tile([128, 512], F32, tag="mm")
                for dc in range(DC):
                    nc.tensor.matmul(pf, w1[:, dc, 128 * fc:128 * (fc + 1)],
                                     h1T[:, dc, :], start=(dc == 0), stop=(dc == DC - 1))
                nc.scalar.activation(gT[:, fc, :], pf, AF.Gelu,
                                     bias=w1b_sb[:, li, fc:fc + 1])
            for m4 in range(4):
                c = 4 * nt + m4
                for eh, (e0, e1) in enumerate(((0, 512), (512, 768))):
                    pf2 = ps.tile([128, 512], F32, tag="mm")
                    for fc in range(FC):
                        nc.tensor.matmul(pf2[:, :e1 - e0],
                                         gT[:, fc, 128 * m4:128 * (m4 + 1)],
                                         w2[:, fc, e0:e1],
                                         start=(fc == 0), stop=(fc == FC - 1))
                    nc.vector.tensor_add(h[:, c, e0:e1], pf2[:, :e1 - e0],
                                         h[:, c, e0:e1])
                    if w2b_bc is not None:
                        nc.vector.tensor_add(h[:, c, e0:e1], h[:, c, e0:e1],
                                             w2b_bc[:, e0:e1])
                ln_chunk(c, g2_bc, b2_bc)

    # ---- output: first 64 tokens (OCR) of each batch element ----
    for b in range(BL):
        nc.sync.dma_start(out_d[b], h[0:64, 2 * b, :])
    ctx.close()


def _stage(inputs):
    """Host-side staging: shard + pre-layout. Returns (in_maps, flags)."""
    f32 = np.float32
    ocr = np.asarray(inputs["ocr_feats"], f32)
    obj = np.asarray(inputs["obj_feats"], f32)
    dv = np.concatenate([np.asarray(inputs["ocr_dvs"], f32),
                         np.asarray(inputs["obj_dvs"], f32)], axis=1)[..., 0]  # [B,N]
    x = np.concatenate([ocr, obj], axis=1)  # [B, N, D]

    # weights: [in, out] transposed layouts, bf16
    sem_qkv = np.concatenate([np.asarray(inputs["sa_wq"], f32),
                              np.asarray(inputs["sa_wk"], f32),
                              np.asarray(inputs["sa_wv"], f32)], axis=0)  # [3D, D]
    qkv_w = np.asarray(inputs["qkv_w"], f32)
    wqkvT = np.stack([sem_qkv.T] + [qkv_w[l].T for l in range(L)])  # [5, D, 3D]
    woT = np.stack([np.asarray(inputs["out_w"], f32)[l].T for l in range(L)])
    w1T = np.stack([np.asarray(inputs["ff1_w"], f32)[l].T for l in range(L)])
    w2T = np.stack([np.asarray(inputs["ff2_w"], f32)[l].T for l in range(L)])

    sem_b = np.concatenate([np.asarray(inputs["sa_bq"], f32),
                            np.asarray(inputs["sa_bk"], f32),
                            np.asarray(inputs["sa_bv"], f32)])
    qkvb = np.concatenate([sem_b[None], np.asarray(inputs["qkv_b"], f32)])  # [5,3D]
    qkb_cols = qkvb[:, :2 * D].reshape(L + 1, 12, 128).transpose(0, 2, 1).copy()
    vb_cols = qkvb[:, 2 * D:].reshape(L + 1, 6, 128).transpose(0, 2, 1).copy()
    w1b_cols = (np.asarray(inputs["ff1_b"], f32)
                .reshape(L, FC, 128).transpose(0, 2, 1).copy())

    out_b = np.asarray(inputs["out_b"], f32)   # [L, D]
    ff2_b = np.asarray(inputs["ff2_b"], f32)   # [L, D]
    sem_vb = sem_b[2 * D:]                     # [D]
    bias_rows = np.zeros((1 + 2 * L, D), f32)
    bias_rows[0] = sem_vb
    for l in range(L):
        bias_rows[1 + 2 * l] = out_b[l]
        bias_rows[2 + 2 * l] = ff2_b[l]
    ln_g = np.concatenate([np.asarray(inputs["ln0_g"], f32)[None],
                           np.stack([v for pair in zip(
                               np.asarray(inputs["ln1_g"], f32),
                               np.asarray(inputs["ln2_g"], f32)) for v in pair])])
    ln_b = np.concatenate([np.asarray(inputs["ln0_b"], f32)[None],
                           np.stack([v for pair in zip(
                               np.asarray(inputs["ln1_b"], f32),
                               np.asarray(inputs["ln2_b"], f32)) for v in pair])])

    flags = {
        "qkb": bool(np.any(qkvb[:, :2 * D] != 0)),
        "sem_vb": bool(np.any(sem_vb != 0)),
        "wob": bool(np.any(out_b != 0)),
        "w2b": bool(np.any(ff2_b != 0)),
        "lngb": bool(np.any(ln_g != 1) or np.any(ln_b != 0)),
    }

    shared = {
        "wqkvT": wqkvT.astype(ml_dtypes.bfloat16),
        "woT": woT.astype(ml_dtypes.bfloat16),
        "w1T": w1T.astype(ml_dtypes.bfloat16),
        "w2T": w2T.astype(ml_dtypes.bfloat16),
        "qkb_cols": qkb_cols, "vb_cols": vb_cols, "w1b_cols": w1b_cols,
        "bias_rows": bias_rows, "ln_g": ln_g, "ln_b": ln_b,
    }
    in_maps = []
    for c in range(NCORES):
        xs = x[c * BL:(c + 1) * BL].reshape(T, D)
        dvs = dv[c * BL:(c + 1) * BL]  # [BL, N]
        in_maps.append(dict(
            shared,
            x=np.ascontiguousarray(xs),
            xT=np.ascontiguousarray(xs.T).astype(ml_dtypes.bfloat16),
            dv_rows=np.ascontiguousarray(
                np.broadcast_to(dvs[None], (128, BL, N))).copy(),
            dv_cols=np.ascontiguousarray(
                dvs.reshape(BL, 2, 128).transpose(2, 0, 1).reshape(128, 2 * BL)),
        ))
    return in_maps, flags


_CACHE = {}


def _get_nc(flags):
    key = tuple(sorted(flags.items()))
    if key not in _CACHE:
        _CACHE[key] = _build(flags)
    return _CACHE[key]


def kernel(**inputs):
    in_maps, flags = _stage(inputs)
    nc = _get_nc(flags)
    res = run_bass_kernel_spmd(nc, in_maps, list(range(NCORES)))
    outs = [res.results[c]["out"] for c in range(NCORES)]  # each [BL, 64, D]
    return np.concatenate(outs, axis=0).astype(np.float32)


# revision 24
# speedup vs baseline: 1.3683x; 1.3683x over previous
"""DeFUM dense-transformer kernel for 8x Trainium2 NeuronCores.

Sharding: data-parallel over batch B=64 -> 8 batch elements per core.
Each core runs the full network (semantic attention + 4 encoder layers)
on its shard; no collectives. Host does layout staging only (concat,
transpose, dtype cast); all FLOPs happen on device.

Math (per core, BL=8, N=256 tokens, D=768, H=12, DH=64, DFF=2048):
  x   = concat(ocr, obj)                      [2048, 768]  (token-major, f32)
  R   = |log dv_i - log dv_j| per batch       bias for semantic attention
  sem: q,k,v = x@Wq.T etc; att = softmax(qk/sqrt(D)+R); h = LN(x + att@v)
  4x post-norm encoder layers (MHA + exact-gelu FFN)
  out = h[:, :64, :]

Layouts: residual stream h kept f32, token-major [128, 16, 768] in SBUF,
updated in place. Matmul operands are cast to bf16 at PSUM eviction
(fp32 accumulation in PSUM). Weights are host-pre-transposed to [in, out]
so every matmul streams contiguous DMA; activation transposes (h -> h^T,
att -> att^T) run on the PE in bf16.
"""

import numpy as np
import ml_dtypes

import concourse.bass as bass
import concourse.mybir as mybir
import concourse.tile as tile
from concourse.bass_utils import run_bass_kernel_spmd
from concourse.masks import make_identity

B, N_OCR, N_OBJ, D, H, L, DFF = 64, 64, 192, 768, 12, 4, 2048
N = N_OCR + N_OBJ            # 256 tokens per batch element
DH = D // H                  # 64
NCORES = 8
BL = B // NCORES             # 8 batch elements per core
T = BL * N                   # 2048 tokens per core
TC = T // 128                # 16 token chunks
DC = D // 128                # 6 d chunks
FC = DFF // 128              # 16 dff chunks
EPS = 1e-5
AF = mybir.ActivationFunctionType
ALU = mybir.AluOpType
F32 = mybir.dt.float32
BF16 = mybir.dt.bfloat16


def _build(flags):
    """Build the SPMD Bass program for one core."""
    nc = bass.Bass()

    # ---- DRAM parameters (host-staged layouts) ----
    x_d = nc.declare_dram_parameter("x", [T, D], F32, isOutput=False)
    xT_d = nc.declare_dram_parameter("xT", [D, T], BF16, isOutput=False)
    dvr_d = nc.declare_dram_parameter("dv_rows", [128, BL, N], F32, isOutput=False)
    dvc_d = nc.declare_dram_parameter("dv_cols", [128, 2 * BL], F32, isOutput=False)
    wqkvT_d = nc.declare_dram_parameter("wqkvT", [L + 1, D, 3 * D], BF16, False)
    woT_d = nc.declare_dram_parameter("woT", [L, D, D], BF16, False)
    w1T_d = nc.declare_dram_parameter("w1T", [L, D, DFF], BF16, False)
    w2T_d = nc.declare_dram_parameter("w2T", [L, DFF, D], BF16, False)
    qkb_d = nc.declare_dram_parameter("qkb_cols", [L + 1, 128, 12], F32, False)
    vb_d = nc.declare_dram_parameter("vb_cols", [L + 1, 128, 6], F32, False)
    w1b_d = nc.declare_dram_parameter("w1b_cols", [L, 128, FC], F32, False)
    # free-dim bias/ln rows (used only when the matching flag is set)
    brow_d = nc.declare_dram_parameter("bias_rows", [1 + 2 * L, D], F32, False)
    lng_d = nc.declare_dram_parameter("ln_g", [1 + 2 * L, D], F32, False)
    lnb_d = nc.declare_dram_parameter("ln_b", [1 + 2 * L, D], F32, False)
    out_d = nc.declare_dram_parameter("out", [BL, N_OCR, D], F32, isOutput=True)

    def bcast_ap(src_ap, parts=128):
        """Broadcast a 1-partition AP across `parts` partitions (stride 0)."""
        return bass.AP(
            tensor=src_ap.tensor,
            offset=src_ap.offset,
            ap=[[0, parts]] + list(src_ap.ap),
        )

    with tile.TileContext(nc) as tc_:
        _emit(nc, tc_, flags, locals())
    _split_multi_waits(nc)
    return nc


def _split_multi_waits(nc):
    """This toolchain's walrus accepts at most ONE semaphore wait per
    instruction (64B ISA has a single wait-event slot and this build refuses
    to legalize). Hoist extra waits emitted by Tile onto injected NOPs on the
    same engine, placed immediately before the instruction — the engine
    stalls on the NOPs first, so semantics are identical."""
    import bass_rust

    n_new = 0
    for bb in nc.main_func.blocks:
        out = []
        changed = False
        for inst in bb.instructions:
            si = inst.sync_info
            waits = list(si.on_wait) if si is not None and si.on_wait else []
            if len(waits) > 1:
                changed = True
                for w in waits[:-1]:
                    nop = bass_rust.InstNoOp(name=f"I-wsplit-{n_new}")
                    n_new += 1
                    nop.engine = inst.engine
                    nop.sync_info = bass_rust.SyncInfo(on_wait=[w], on_update=[])
                    nc.register_instruction(nop)
                    out.append(nop)
                inst.sync_info = bass_rust.SyncInfo(
                    on_wait=[waits[-1]],
                    on_update=list(si.on_update) if si.on_update else [],
                )
            out.append(inst)
        if changed:
            bb.instructions = out
    return n_new


def _emit(nc, tc, flags, d):
    x_d, xT_d, dvr_d, dvc_d = d["x_d"], d["xT_d"], d["dvr_d"], d["dvc_d"]
    wqkvT_d, woT_d, w1T_d, w2T_d = d["wqkvT_d"], d["woT_d"], d["w1T_d"], d["w2T_d"]
    qkb_d, vb_d, w1b_d = d["qkb_d"], d["vb_d"], d["w1b_d"]
    brow_d, lng_d, lnb_d, out_d = d["brow_d"], d["lng_d"], d["lnb_d"], d["out_d"]
    bcast_ap = d["bcast_ap"]

    from contextlib import ExitStack
    ctx = ExitStack()
    const = ctx.enter_context(tc.tile_pool(name="const", bufs=1))
    wpool = ctx.enter_context(tc.tile_pool(name="w", bufs=1))
    act = ctx.enter_context(tc.tile_pool(name="act", bufs=1))
    wk1 = ctx.enter_context(tc.tile_pool(name="wk1", bufs=1))
    wk2 = ctx.enter_context(tc.tile_pool(name="wk2", bufs=2))
    ps = ctx.enter_context(tc.tile_pool(name="ps", bufs=3, space="PSUM"))
    psav = ctx.enter_context(tc.tile_pool(name="psav", bufs=2, space="PSUM"))
    pst = ctx.enter_context(tc.tile_pool(name="pst", bufs=1, space="PSUM"))
    psum_s = ctx.enter_context(tc.tile_pool(name="psum_s", bufs=2, space="PSUM"))

    # ---- constants ----
    ident = const.tile([128, 128], BF16)
    make_identity(nc, ident)
    ones_bf = const.tile([128, 128], BF16)
    nc.vector.memset(ones_bf, 1.0)
    eps_t = const.tile([128, 1], F32)
    nc.vector.memset(eps_t, EPS)
    qkb_sb = const.tile([128, L + 1, 12], F32)
    nc.sync.dma_start(qkb_sb, qkb_d[:].rearrange("l p c -> p l c"))
    vb_sb = const.tile([128, L + 1, 6], F32)
    nc.sync.dma_start(vb_sb, vb_d[:].rearrange("l p c -> p l c"))
    w1b_sb = const.tile([128, L, FC], F32)
    nc.sync.dma_start(w1b_sb, w1b_d[:].rearrange("l p c -> p l c"))
    ldvc = const.tile([128, 2 * BL], F32)
    nc.sync.dma_start(ldvc, dvc_d[:])
    nc.scalar.activation(ldvc, ldvc, AF.Ln)
    # log(dv) rows, pre-broadcast across partitions on host
    ldvr_all = const.tile([128, BL, N], F32)
    nc.sync.dma_start(ldvr_all, dvr_d[:])
    nc.scalar.activation(ldvr_all, ldvr_all, AF.Ln)

    # residual stream, in-place across all phases
    h = act.tile([128, TC, D], F32)
    nc.sync.dma_start(h, x_d[:].rearrange("(c p) d -> p c d", p=128))

    # ---- helpers ----
    def ln_chunk(c, g_bc=None, b_bc=None):
        """In-place LayerNorm of h[:, c, :] along the free (D) axis."""
        stats = wk2.tile([128, 3, 6], F32, tag="bnst")
        for i in range(3):
            nc.vector.bn_stats(stats[:, i, :], h[:, c, 256 * i:256 * (i + 1)])
        mv = wk2.tile([128, 2], F32, tag="bnmv")
        nc.vector.bn_aggr(mv, stats)
        # rstd = exp(-0.5*ln(var+eps)): Ln and Exp share one ACT table,
        # avoiding Sqrt-table reloads between softmax and LN
        rs = wk2.tile([128, 1], F32, tag="bnrs")
        nc.scalar.activation(rs, mv[:, 1:2], AF.Ln, bias=eps_t)
        nc.scalar.activation(rs, rs, AF.Exp, scale=-0.5)
        nc.vector.tensor_scalar(
            out=h[:, c, :], in0=h[:, c, :],
            scalar1=mv[:, 0:1], scalar2=rs,
            op0=ALU.subtract, op1=ALU.mult,
        )
        if g_bc is not None:
            nc.vector.tensor_mul(h[:, c, :], h[:, c, :], g_bc)
        if b_bc is not None:
            nc.vector.tensor_add(h[:, c, :], h[:, c, :], b_bc)

    def load_brow(tag, src_ap):
        t = wk2.tile([128, D], F32, tag=tag)
        nc.gpsimd.dma_start(t, bcast_ap(src_ap))
        return t

    def exp_scores(ps_scT, li, R=None):
        """exp of transposed scores psum [128(j), 2(jc), 256(i)] -> bf16,
        UNNORMALIZED (softmax denominator applied downstream)."""
        expT = wk2.tile([128, 2, N], BF16, tag="expT")
        scale = 1.0 / np.sqrt(np.float32(D if li == 0 else DH))
        if R is not None:
            lg = wk1.tile([128, 2, N], F32, tag="logit")
            nc.vector.scalar_tensor_tensor(
                out=lg, in0=ps_scT[:, :, :], scalar=float(scale),
                in1=R[:, :, :], op0=ALU.mult, op1=ALU.add,
            )
            nc.scalar.activation(expT[:, :, :], lg, AF.Exp)
        else:
            nc.scalar.activation(expT[:, :, :], ps_scT[:, :, :], AF.Exp,
                                 scale=float(scale))
        return expT

    # ================= semantic attention (layer index 0 in wqkvT) ========
    wqkv = wpool.tile([128, DC, 3 * D], BF16, tag="wqkv")
    nc.sync.dma_start(wqkv, wqkvT_d[0].rearrange("(c p) o -> p c o", p=128))

    sem_vb_bc = load_brow("sem_vb", brow_d[0]) if flags["sem_vb"] else None
    ln0_g_bc = load_brow("lng", lng_d[0]) if flags["lngb"] else None
    ln0_b_bc = load_brow("lnb", lnb_d[0]) if flags["lngb"] else None

    for b in range(BL):
        xT_b = wk1.tile([128, DC, N], BF16, tag="hT")
        nc.sync.dma_start(
            xT_b, xT_d[:].rearrange("(c p) t -> p c t", p=128)[:, :, N * b:N * (b + 1)])
        # q^T, k^T  [e-chunk, token] ; v [token, e]
        qkT = wk2.tile([128, 12, N], BF16, tag="qkT")
        for op2 in range(6):
            pq = ps.tile([128, 512], F32, tag="mm")
            for half in range(2):
                oc = 2 * op2 + half
                for dc in range(DC):
                    nc.tensor.matmul(pq[:, N * half:N * (half + 1)],
                                     wqkv[:, dc, 128 * oc:128 * (oc + 1)],
                                     xT_b[:, dc, :], start=(dc == 0),
                                     stop=(dc == DC - 1))
            if flags["qkb"]:
                for half in range(2):
                    oc = 2 * op2 + half
                    nc.vector.tensor_scalar_add(qkT[:, oc, :],
                                                pq[:, N * half:N * (half + 1)],
                                                qkb_sb[:, 0, oc:oc + 1])
            else:
                nc.vector.tensor_copy(qkT[:, 2 * op2:2 * op2 + 2, :], pq)
        v_b = wk2.tile([128, 2, D], BF16, tag="v")
        for tc2 in range(2):
            for ev in range(2):
                pv = ps.tile([128, 512], F32, tag="mm")
                for dc in range(DC):
                    nc.tensor.matmul(
                        pv[:, :384],
                        xT_b[:, dc, 128 * tc2:128 * (tc2 + 1)],
                        wqkv[:, dc, 2 * D + 384 * ev:2 * D + 384 * (ev + 1)],
                        start=(dc == 0), stop=(dc == DC - 1))
                nc.vector.tensor_copy(v_b[:, tc2, 384 * ev:384 * (ev + 1)],
                                      pv[:, :384])
        # R bias
        R = wk1.tile([128, 2, N], F32, tag="R")
        for ic in range(2):
            nc.vector.tensor_scalar(
                out=R[:, ic, :], in0=ldvr_all[:, b, :],
                scalar1=ldvc[:, 2 * b + ic:2 * b + ic + 1],
                scalar2=None, op0=ALU.subtract)
            nc.scalar.activation(R[:, ic, :], R[:, ic, :], AF.Abs)
        # transposed scores: scT[j, i] = sum_e kT[e,j] qT[e,i]
        ps_scT = ps.tile([128, 2, N], F32, tag="mm")
        for jc in range(2):
            for dc in range(DC):
                nc.tensor.matmul(ps_scT[:, jc, :],
                                 qkT[:, 6 + dc, 128 * jc:128 * (jc + 1)],
                                 qkT[:, dc, :],
                                 start=(dc == 0), stop=(dc == DC - 1))
        expT = exp_scores(ps_scT, 0, R=R)
        # softmax denominators as a column [i, 1]: lhsT=expT, rhs=ones
        ps_cs = psum_s.tile([128, 2], F32, tag="sums")
        for ic in range(2):
            for jc in range(2):
                nc.tensor.matmul(ps_cs[:, ic:ic + 1],
                                 expT[:, jc, 128 * ic:128 * (ic + 1)],
                                 ones_bf[:, 0:1],
                                 start=(jc == 0), stop=(jc == 1))
        rec2 = wk2.tile([128, 2], F32, tag="rsum2")
        nc.vector.reciprocal(rec2, ps_cs)
        # av (canonical [token, e]); normalization + residual fused in evict
        for ic in range(2):
            for ev in range(2):
                pa = ps.tile([128, 512], F32, tag="mm")
                for jc in range(2):
                    nc.tensor.matmul(pa[:, :384],
                                     expT[:, jc, 128 * ic:128 * (ic + 1)],
                                     v_b[:, jc, 384 * ev:384 * (ev + 1)],
                                     start=(jc == 0), stop=(jc == 1))
                c = 2 * b + ic
                sl = slice(384 * ev, 384 * (ev + 1))
                nc.vector.scalar_tensor_tensor(
                    out=h[:, c, sl], in0=pa[:, :384], scalar=rec2[:, ic:ic + 1],
                    in1=h[:, c, sl], op0=ALU.mult, op1=ALU.add)
                if sem_vb_bc is not None:
                    nc.vector.tensor_add(h[:, c, sl], h[:, c, sl], sem_vb_bc[:, sl])
        for ic in range(2):
            ln_chunk(2 * b + ic, ln0_g_bc, ln0_b_bc)

    # ================= encoder layers =================
    for li in range(L):
        wqkv = wpool.tile([128, DC, 3 * D], BF16, tag="wqkv")
        nc.sync.dma_start(wqkv, wqkvT_d[li + 1].rearrange("(c p) o -> p c o", p=128))
        wo = wpool.tile([128, DC, D], BF16, tag="wo")
        nc.sync.dma_start(wo, woT_d[li].rearrange("(c p) o -> p c o", p=128))
        w1 = wpool.tile([128, DC, DFF], BF16, tag="w1")
        nc.sync.dma_start(w1, w1T_d[li].rearrange("(c p) o -> p c o", p=128))
        w2 = wpool.tile([128, FC, D], BF16, tag="w2")
        nc.sync.dma_start(w2, w2T_d[li].rearrange("(c p) o -> p c o", p=128))

        wob_bc = load_brow("wob", brow_d[1 + 2 * li]) if flags["wob"] else None
        w2b_bc = load_brow("w2b", brow_d[2 + 2 * li]) if flags["w2b"] else None
        g1_bc = load_brow("lng", lng_d[1 + 2 * li]) if flags["lngb"] else None
        b1_bc = load_brow("lnb", lnb_d[1 + 2 * li]) if flags["lngb"] else None
        g2_bc = load_brow("lng2", lng_d[2 + 2 * li]) if flags["lngb"] else None
        b2_bc = load_brow("lnb2", lnb_d[2 + 2 * li]) if flags["lngb"] else None

        # ---- attention, one batch element at a time ----
        for b in range(BL):
            hcast = wk1.tile([128, 4, D], BF16, tag="hcast")
            for ic in range(2):
                nc.vector.tensor_copy(hcast[:, ic, :], h[:, 2 * b + ic, :])
            hT_b = wk1.tile([128, DC, N], BF16, tag="hT")
            for dc in range(DC):
                tp = pst.tile([128, 512], BF16, tag="tr")
                for ic in range(2):
                    nc.tensor.transpose(tp[:, 128 * ic:128 * (ic + 1)],
                                        hcast[:, ic, 128 * dc:128 * (dc + 1)], ident)
                nc.vector.tensor_copy(hT_b[:, dc, :], tp[:, :N])
            qkT = wk2.tile([128, 12, N], BF16, tag="qkT")
            for op2 in range(6):
                pq = ps.tile([128, 512], F32, tag="mm")
                for half in range(2):
                    oc = 2 * op2 + half
                    for dc in range(DC):
                        nc.tensor.matmul(pq[:, N * half:N * (half + 1)],
                                         wqkv[:, dc, 128 * oc:128 * (oc + 1)],
                                         hT_b[:, dc, :], start=(dc == 0),
                                         stop=(dc == DC - 1))
                if flags["qkb"]:
                    for half in range(2):
                        oc = 2 * op2 + half
                        nc.vector.tensor_scalar_add(
                            qkT[:, oc, :], pq[:, N * half:N * (half + 1)],
                            qkb_sb[:, li + 1, oc:oc + 1])
                else:
                    nc.vector.tensor_copy(qkT[:, 2 * op2:2 * op2 + 2, :], pq)
            v_b = wk2.tile([128, 2, D], BF16, tag="v")
            for tc2 in range(2):
                for ev in range(2):
                    pv = ps.tile([128, 512], F32, tag="mm")
                    for dc in range(DC):
                        nc.tensor.matmul(
                            pv[:, :384],
                            hT_b[:, dc, 128 * tc2:128 * (tc2 + 1)],
                            wqkv[:, dc, 2 * D + 384 * ev:2 * D + 384 * (ev + 1)],
                            start=(dc == 0), stop=(dc == DC - 1))
                    nc.vector.tensor_copy(v_b[:, tc2, 384 * ev:384 * (ev + 1)],
                                          pv[:, :384])
            # per-head attention; accumulate attn_out^T [e-chunk, token].
            # av consumes UNNORMALIZED exp; per-(head,query) denominators are
            # summed into psum rows via ones-matmuls (m=64 half per head) and
            # applied as 1/sum at eviction (recip = Exp(-Ln(sums)) on ACT).
            aoT = wk2.tile([128, DC, N], BF16, tag="aoT")
            for ec2 in range(3):
                pav = psav.tile([128, 2, N], F32, tag="av")
                ps_sums = psum_s.tile([128, 2, N], F32, tag="sums")
                for ecs in range(2):
                    ec = 2 * ec2 + ecs
                    for sub in range(2):
                        hd, off = 2 * ec + sub, sub * 64
                        ps_scT = ps.tile([128, 2, N], F32, tag="mm")
                        for jc in range(2):
                            nc.tensor.matmul(
                                ps_scT[:, jc, :],
                                qkT[off:off + 64, 6 + ec, 128 * jc:128 * (jc + 1)],
                                qkT[off:off + 64, ec, :],
                                start=True, stop=True)
                        expT = exp_scores(ps_scT, li + 1)
                        for jc in range(2):
                            nc.tensor.matmul(
                                ps_sums[off:off + 64, ecs, :],
                                ones_bf[:, 0:64], expT[:, jc, :],
                                start=(jc == 0), stop=(jc == 1))
                        for jc in range(2):
                            nc.tensor.matmul(
                                pav[off:off + 64, ecs, :],
                                v_b[:, jc, DH * hd:DH * (hd + 1)],
                                expT[:, jc, :],
                                start=(jc == 0), stop=(jc == 1))
                rec = wk2.tile([128, 2, N], F32, tag="rsum")
                nc.scalar.activation(rec[:, :, :], ps_sums[:, :, :], AF.Ln)
                nc.scalar.activation(rec[:, :, :], rec[:, :, :], AF.Exp,
                                     scale=-1.0)
                nc.vector.tensor_tensor(
                    out=aoT[:, 2 * ec2:2 * ec2 + 2, :], in0=pav[:, :, :],
                    in1=rec[:, :, :], op=ALU.mult)
                if flags["vb"]:
                    for ecs in range(2):
                        nc.vector.tensor_scalar_add(
                            aoT[:, 2 * ec2 + ecs, :], aoT[:, 2 * ec2 + ecs, :],
                            vb_sb[:, li + 1, 2 * ec2 + ecs:2 * ec2 + ecs + 1])
            # out-projection + residual into h, then LN1
            for ic in range(2):
                c = 2 * b + ic
                for fh in range(2):
                    po = ps.tile([128, 512], F32, tag="mm")
                    for ec in range(DC):
                        nc.tensor.matmul(po[:, :384],
                                         aoT[:, ec, 128 * ic:128 * (ic + 1)],
                                         wo[:, ec, 384 * fh:384 * (fh + 1)],
                                         start=(ec == 0), stop=(ec == DC - 1))
                    sl = slice(384 * fh, 384 * (fh + 1))
                    nc.vector.tensor_add(h[:, c, sl], po[:, :384], h[:, c, sl])
                    if wob_bc is not None:
                        nc.vector.tensor_add(h[:, c, sl], h[:, c, sl], wob_bc[:, sl])
                ln_chunk(c, g1_bc, b1_bc)

        # ---- FFN over 512-token chunks ----
        for nt in range(4):
            h1cast = wk1.tile([128, 4, D], BF16, tag="hcast")
            for t4 in range(4):
                nc.vector.tensor_copy(h1cast[:, t4, :], h[:, 4 * nt + t4, :])
            h1T = wk1.tile([128, DC, 512], BF16, tag="h1T")
            for dc in range(DC):
                tp = pst.tile([128, 512], BF16, tag="tr")
                for t4 in range(4):
                    nc.tensor.transpose(tp[:, 128 * t4:128 * (t4 + 1)],
                                        h1cast[:, t4, 128 * dc:128 * (dc + 1)], ident)
                nc.vector.tensor_copy(h1T[:, dc, :], tp)
            gT = wk1.tile([128, FC, 512], BF16, tag="gT")
            for fc in range(FC):
                pf = ps.tile([128, 512], F32, tag="mm")
                for dc in range(DC):
                    nc.tensor.matmul(pf, w1[:, dc, 128 * fc:128 * (fc + 1)],
                                     h1T[:, dc, :], start=(dc == 0), stop=(dc == DC - 1))
                nc.scalar.activation(gT[:, fc, :], pf, AF.Gelu,
                                     bias=w1b_sb[:, li, fc:fc + 1])
            for m4 in range(4):
                c = 4 * nt + m4
                for eh, (e0, e1) in enumerate(((0, 512), (512, 768))):
                    pf2 = ps.tile([128, 512], F32, tag="mm")
                    for fc in range(FC):
                        nc.tensor.matmul(pf2[:, :e1 - e0],
                                         gT[:, fc, 128 * m4:128 * (m4 + 1)],
                                         w2[:, fc, e0:e1],
                                         start=(fc == 0), stop=(fc == FC - 1))
                    nc.vector.tensor_add(h[:, c, e0:e1], pf2[:, :e1 - e0],
                                         h[:, c, e0:e1])
                    if w2b_bc is not None:
                        nc.vector.tensor_add(h[:, c, e0:e1], h[:, c, e0:e1],
                                             w2b_bc[:, e0:e1])
                ln_chunk(c, g2_bc, b2_bc)

    # ---- output: first 64 tokens (OCR) of each batch element ----
    for b in range(BL):
        nc.sync.dma_start(out_d[b], h[0:64, 2 * b, :])
    ctx.close()


def _stage(inputs):
    """Host-side staging: shard + pre-layout. Returns (in_maps, flags)."""
    f32 = np.float32
    ocr = np.asarray(inputs["ocr_feats"], f32)
    obj = np.asarray(inputs["obj_feats"], f32)
    dv = np.concatenate([np.asarray(inputs["ocr_dvs"], f32),
                         np.asarray(inputs["obj_dvs"], f32)], axis=1)[..., 0]  # [B,N]
    x = np.concatenate([ocr, obj], axis=1)  # [B, N, D]

    # weights: [in, out] transposed layouts, bf16
    sem_qkv = np.concatenate([np.asarray(inputs["sa_wq"], f32),
                              np.asarray(inputs["sa_wk"], f32),
                              np.asarray(inputs["sa_wv"], f32)], axis=0)  # [3D, D]
    qkv_w = np.asarray(inputs["qkv_w"], f32)
    wqkvT = np.stack([sem_qkv.T] + [qkv_w[l].T for l in range(L)])  # [5, D, 3D]
    woT = np.stack([np.asarray(inputs["out_w"], f32)[l].T for l in range(L)])
    w1T = np.stack([np.asarray(inputs["ff1_w"], f32)[l].T for l in range(L)])
    w2T = np.stack([np.asarray(inputs["ff2_w"], f32)[l].T for l in range(L)])

    sem_b = np.concatenate([np.asarray(inputs["sa_bq"], f32),
                            np.asarray(inputs["sa_bk"], f32),
                            np.asarray(inputs["sa_bv"], f32)])
    qkvb = np.concatenate([sem_b[None], np.asarray(inputs["qkv_b"], f32)])  # [5,3D]
    qkb_cols = qkvb[:, :2 * D].reshape(L + 1, 12, 128).transpose(0, 2, 1).copy()
    vb_cols = qkvb[:, 2 * D:].reshape(L + 1, 6, 128).transpose(0, 2, 1).copy()
    w1b_cols = (np.asarray(inputs["ff1_b"], f32)
                .reshape(L, FC, 128).transpose(0, 2, 1).copy())

    out_b = np.asarray(inputs["out_b"], f32)   # [L, D]
    ff2_b = np.asarray(inputs["ff2_b"], f32)   # [L, D]
    sem_vb = sem_b[2 * D:]                     # [D]
    bias_rows = np.zeros((1 + 2 * L, D), f32)
    bias_rows[0] = sem_vb
    for l in range(L):
        bias_rows[1 + 2 * l] = out_b[l]
        bias_rows[2 + 2 * l] = ff2_b[l]
    ln_g = np.concatenate([np.asarray(inputs["ln0_g"], f32)[None],
                           np.stack([v for pair in zip(
                               np.asarray(inputs["ln1_g"], f32),
                               np.asarray(inputs["ln2_g"], f32)) for v in pair])])
    ln_b = np.concatenate([np.asarray(inputs["ln0_b"], f32)[None],
                           np.stack([v for pair in zip(
                               np.asarray(inputs["ln1_b"], f32),
                               np.asarray(inputs["ln2_b"], f32)) for v in pair])])

    flags = {
        "qkb": bool(np.any(qkvb[:, :2 * D] != 0)),
        "vb": bool(np.any(qkvb[1:, 2 * D:] != 0)),
        "sem_vb": bool(np.any(sem_vb != 0)),
        "wob": bool(np.any(out_b != 0)),
        "w2b": bool(np.any(ff2_b != 0)),
        "lngb": bool(np.any(ln_g != 1) or np.any(ln_b != 0)),
    }

    shared = {
        "wqkvT": wqkvT.astype(ml_dtypes.bfloat16),
        "woT": woT.astype(ml_dtypes.bfloat16),
        "w1T": w1T.astype(ml_dtypes.bfloat16),
        "w2T": w2T.astype(ml_dtypes.bfloat16),
        "qkb_cols": qkb_cols, "vb_cols": vb_cols, "w1b_cols": w1b_cols,
        "bias_rows": bias_rows, "ln_g": ln_g, "ln_b": ln_b,
    }
    in_maps = []
    for c in range(NCORES):
        xs = x[c * BL:(c + 1) * BL].reshape(T, D)
        dvs = dv[c * BL:(c + 1) * BL]  # [BL, N]
        in_maps.append(dict(
            shared,
            x=np.ascontiguousarray(xs),
            xT=np.ascontiguousarray(xs.T).astype(ml_dtypes.bfloat16),
            dv_rows=np.ascontiguousarray(
                np.broadcast_to(dvs[None], (128, BL, N))).copy(),
            dv_cols=np.ascontiguousarray(
                dvs.reshape(BL, 2, 128).transpose(2, 0, 1).reshape(128, 2 * BL)),
        ))
    return in_maps, flags


_CACHE = {}


def _get_nc(flags):
    key = tuple(sorted(flags.items()))
    if key not in _CACHE:
        _CACHE[key] = _build(flags)
    return _CACHE[key]


def kernel(**inputs):
    in_maps, flags = _stage(inputs)
    nc = _get_nc(flags)
    res = run_bass_kernel_spmd(nc, in_maps, list(range(NCORES)))
    outs = [res.results[c]["out"] for c in range(NCORES)]  # each [BL, 64, D]
    return np.concatenate(outs, axis=0).astype(np.float32)
